# revision 3
# baseline (speedup 1.0000x reference)
"""Trainium2 Bass kernel for DiffCompressModule.

Reference computation (B=4, S=512, D_IN=D_OUT=4096):
    out = h @ W.T + b + coeff[b] * (h @ (2*mask[b] - 1))

Fused form used here (one matmul instead of two):
    out[b] = h[b] @ M_b + bias,   M_b = W.T + coeff[b] * (2*mask[b] - 1)

M_b is built in bf16 on ACT+DVE while the 256MB int32 mask streams from
HBM; the matmul runs in bf16 with fp32 PSUM accumulation. The kernel is
HBM-bound (~68MB per core).

Sharding over 8 cores: 4 out-feature groups x 2 batch groups.
Each core: h [2,512,4096], W [1024,4096], bias [1024], coeff [2],
mask [2,4096,1024] -> out [2,512,1024].
"""

import numpy as np

import concourse.bass as bass
import concourse.mybir as mybir
from concourse import tile, masks
from concourse.bass_utils import run_bass_kernel_spmd

B, S, D = 4, 512, 4096
O_FULL = 4096
N_CORES = 8
OG, BG = 4, 2  # out-feature groups x batch groups
O_SH = O_FULL // OG  # 1024 out features per core
B_SH = B // BG  # 2 batches per core
HALF = 512  # o processed in halves (PSUM/SBUF budget)
KC = D // 128  # 32 contraction chunks
SC = S // 128  # 4 s chunks
dt = mybir.dt

_CACHE = {}


def _split_sync_waits(nc, max_waits=1):
    # CoreV3 walrus rejects instructions with several semaphore waits
    # ("Too many sync wait commands") - notably Tile's kernel-tail drain.
    # Splitting the waits across preceding same-engine NOPs is equivalent.
    ctr = 0
    for fn in nc.m.functions:
        for bb in fn.blocks:
            insts = bb.instructions
            if not any(
                i.sync_info is not None and len(i.sync_info.on_wait) > max_waits
                for i in insts
            ):
                continue
            new_list = []
            for ins in insts:
                si = ins.sync_info
                if si is not None and len(si.on_wait) > max_waits:
                    waits = list(si.on_wait)
                    head, tail = waits[:-max_waits], waits[-max_waits:]
                    for k in range(0, len(head), max_waits):
                        nop = mybir.InstNoOp(
                            name=f"waitsplit-{ctr}",
                            engine=ins.engine,
                            ins=[],
                            outs=[],
                            sync_info=mybir.SyncInfo(
                                on_wait=head[k : k + max_waits], on_update=[]
                            ),
                        )
                        ctr += 1
                        new_list.append(nop)
                    ins.sync_info = mybir.SyncInfo(
                        on_wait=tail, on_update=list(si.on_update)
                    )
                new_list.append(ins)
            bb.instructions = new_list


def _build_nc():
    nc = bass.Bass("TRN2", target_bir_lowering=False, debug=False)
    h = nc.dram_tensor("h", [B_SH, S, D], dt.float32, kind="ExternalInput").ap()
    W = nc.dram_tensor("W", [O_SH, D], dt.float32, kind="ExternalInput").ap()
    bias = nc.dram_tensor("bias", [O_SH], dt.float32, kind="ExternalInput").ap()
    coeff = nc.dram_tensor("coeff", [B_SH], dt.float32, kind="ExternalInput").ap()
    mask = nc.dram_tensor("mask", [B_SH, D, O_SH], dt.int32, kind="ExternalInput").ap()
    out = nc.dram_tensor("out", [B_SH, S, O_SH], dt.float32, kind="ExternalOutput").ap()

    with tile.TileContext(nc) as tc:
        with (
            tc.tile_pool(name="const", bufs=1) as const_pool,
            tc.tile_pool(name="stage", bufs=6) as stage_pool,  # h/W f32 staging
            tc.tile_pool(name="wt", bufs=KC + 2) as wt_pool,
            tc.tile_pool(name="ht", bufs=B_SH * KC) as ht_pool,
            tc.tile_pool(name="mk", bufs=8) as mk_pool,
            tc.tile_pool(name="tt", bufs=3) as t_pool,
            tc.tile_pool(name="m", bufs=KC + 4) as m_pool,
            tc.tile_pool(name="ost", bufs=3) as out_pool,
            tc.tile_pool(name="tp", bufs=2, space="PSUM") as tp_pool,
            tc.tile_pool(name="acc", bufs=4, space="PSUM") as acc_pool,
        ):
            ident = const_pool.tile([128, 128], dt.float32)
            masks.make_identity(nc, ident[:])

            bias_bc = const_pool.tile([128, O_SH], dt.float32)
            nc.sync.dma_start(
                bias_bc[:], bass.AP(bias.tensor, 0, [[0, 128], [1, O_SH]])
            )
            coeff_bc = const_pool.tile([128, B_SH], dt.float32)
            nc.sync.dma_start(
                coeff_bc[:], bass.AP(coeff.tensor, 0, [[0, 128], [1, B_SH]])
            )
            c2 = const_pool.tile([128, B_SH], dt.float32)
            cneg = const_pool.tile([128, B_SH], dt.float32)
            nc.vector.tensor_scalar_mul(c2[:], coeff_bc[:], 2.0)
            nc.vector.tensor_scalar_mul(cneg[:], coeff_bc[:], -1.0)

            # --- hT: h [b, s, i] -> per (b, kc) tiles [128 i, 512 s] bf16 ---
            ht = {}
            for b in range(B_SH):
                for kg in range(4):  # groups of 8 kc chunks
                    stg = [
                        stage_pool.tile([128, 1024], dt.float32, tag="stage", name="stg")
                        for _ in range(SC)
                    ]
                    for sc in range(SC):
                        nc.sync.dma_start(
                            stg[sc][:],
                            h[b, sc * 128 : (sc + 1) * 128, kg * 1024 : (kg + 1) * 1024],
                        )
                    for k8 in range(8):
                        kc = kg * 8 + k8
                        tp = tp_pool.tile([128, 512], dt.float32)
                        for sc in range(SC):
                            nc.tensor.transpose(
                                tp[:, sc * 128 : (sc + 1) * 128],
                                stg[sc][:, k8 * 128 : (k8 + 1) * 128],
                                ident[:],
                            )
                        htt = ht_pool.tile([128, S], dt.bfloat16)
                        nc.vector.tensor_copy(htt[:], tp[:])
                        ht[(b, kc)] = htt

            for half in range(2):
                o0 = half * HALF
                # --- WT for this half: [128 i, 512 o] bf16 per kc ---
                wt = []
                for kg in range(4):
                    stg = [
                        stage_pool.tile([128, 1024], dt.float32, tag="stage", name="stg")
                        for _ in range(4)
                    ]
                    for oc in range(4):
                        nc.sync.dma_start(
                            stg[oc][:],
                            W[
                                o0 + oc * 128 : o0 + (oc + 1) * 128,
                                kg * 1024 : (kg + 1) * 1024,
                            ],
                        )
                    for k8 in range(8):
                        kc = kg * 8 + k8
                        tp = tp_pool.tile([128, 512], dt.float32)
                        for oc in range(4):
                            nc.tensor.transpose(
                                tp[:, oc * 128 : (oc + 1) * 128],
                                stg[oc][:, k8 * 128 : (k8 + 1) * 128],
                                ident[:],
                            )
                        wtt = wt_pool.tile([128, HALF], dt.bfloat16)
                        nc.vector.tensor_copy(wtt[:], tp[:])
                        wt.append(wtt)

                for b in range(B_SH):
                    # --- M chunks: T = 2c*mask - c (ACT), M = T + WT (DVE) ---
                    m_tiles = []
                    for kc in range(KC):
                        mk = mk_pool.tile([128, HALF], dt.int32)
                        nc.sync.dma_start(
                            mk[:],
                            mask[b, kc * 128 : (kc + 1) * 128, o0 : o0 + HALF],
                        )
                        t_sb = t_pool.tile([128, HALF], dt.float32)
                        nc.scalar.activation(
                            t_sb[:],
                            mk[:],
                            mybir.ActivationFunctionType.Identity,
                            bias=cneg[:, b : b + 1],
                            scale=c2[:, b : b + 1],
                        )
                        m = m_pool.tile([128, HALF], dt.bfloat16)
                        nc.vector.tensor_tensor(
                            m[:], t_sb[:], wt[kc][:], mybir.AluOpType.add
                        )
                        m_tiles.append(m)

                    # --- matmuls: acc[sc] [128 s, 512 o] += hT.T @ M ---
                    accs = [
                        acc_pool.tile([128, HALF], dt.float32, tag="acc", name="acc")
                        for _ in range(SC)
                    ]
                    for kc in range(KC):
                        for sc in range(SC):
                            nc.tensor.matmul(
                                accs[sc][:],
                                ht[(b, kc)][:, sc * 128 : (sc + 1) * 128],
                                m_tiles[kc][:],
                                start=(kc == 0),
                                stop=(kc == KC - 1),
                            )

                    # --- epilogue: out = acc + bias; DMA out ---
                    for sc in range(SC):
                        o_sb = out_pool.tile([128, HALF], dt.float32)
                        nc.vector.tensor_tensor(
                            o_sb[:],
                            accs[sc][:],
                            bias_bc[:, o0 : o0 + HALF],
                            mybir.AluOpType.add,
                        )
                        nc.sync.dma_start(
                            out[b, sc * 128 : (sc + 1) * 128, o0 : o0 + HALF],
                            o_sb[:],
                        )

    _split_sync_waits(nc)
    return nc


def _get_nc():
    if "nc" not in _CACHE:
        _CACHE["nc"] = _build_nc()
    return _CACHE["nc"]


def kernel(hidden_states, W, b, coeff, mask, _trace=False, _trace_kwargs=None):
    nc = _get_nc()
    in_maps = []
    for core in range(N_CORES):
        g, bj = core // BG, core % BG
        in_maps.append(
            {
                "h": np.ascontiguousarray(
                    hidden_states[bj * B_SH : (bj + 1) * B_SH], dtype=np.float32
                ),
                "W": np.ascontiguousarray(
                    W[g * O_SH : (g + 1) * O_SH], dtype=np.float32
                ),
                "bias": np.ascontiguousarray(
                    b[g * O_SH : (g + 1) * O_SH], dtype=np.float32
                ),
                "coeff": np.ascontiguousarray(
                    coeff[bj * B_SH : (bj + 1) * B_SH], dtype=np.float32
                ),
                "mask": np.ascontiguousarray(
                    mask[bj * B_SH : (bj + 1) * B_SH, :, g * O_SH : (g + 1) * O_SH],
                    dtype=np.int32,
                ),
            }
        )
    kwargs = {}
    if _trace:
        kwargs = {"trace": True, "trace_kwargs": _trace_kwargs or {}}
    res = run_bass_kernel_spmd(nc, in_maps, core_ids=list(range(N_CORES)), **kwargs)
    _CACHE["last_results"] = res

    out = np.empty((B, S, O_FULL), dtype=np.float32)
    for core in range(N_CORES):
        g, bj = core // BG, core % BG
        out[bj * B_SH : (bj + 1) * B_SH, :, g * O_SH : (g + 1) * O_SH] = res.results[
            core
        ]["out"]
    return out


# revision 10
# speedup vs baseline: 1.2085x; 1.2085x over previous
"""Trainium2 Bass kernel for DiffCompressModule.

Reference computation (B=4, S=512, D_IN=D_OUT=4096):
    out = h @ W.T + b + coeff[b] * (h @ (2*mask[b] - 1))

Fused form used here (one matmul instead of two):
    out[b] = h[b] @ M_b + bias,   M_b = W.T + coeff[b] * (2*mask[b] - 1)

M_b is built in bf16 on ACT+DVE while the 256MB int32 mask streams from
HBM; the matmul runs in bf16 with fp32 PSUM accumulation. The kernel is
HBM-bound (~68MB per core).

Sharding over 8 cores: 4 out-feature groups x 2 batch groups.
Each core: h [2,512,4096], W [1024,4096], bias [1024], coeff [2],
mask [2,4096,1024] -> out [2,512,1024].
"""

import numpy as np

import concourse.bass as bass
import concourse.mybir as mybir
from concourse import tile, masks
from concourse.bass_utils import run_bass_kernel_spmd

B, S, D = 4, 512, 4096
O_FULL = 4096
N_CORES = 8
OG, BG = 4, 2  # out-feature groups x batch groups
O_SH = O_FULL // OG  # 1024 out features per core
B_SH = B // BG  # 2 batches per core
HALF = 512  # o processed in halves (PSUM/SBUF budget)
KC = D // 128  # 32 contraction chunks
SC = S // 128  # 4 s chunks
dt = mybir.dt

_CACHE = {}


def _split_sync_waits(nc, max_waits=1):
    # CoreV3 walrus rejects instructions with more than one semaphore wait
    # ("Too many sync wait commands"). Splitting the waits across preceding
    # same-engine NOPs is equivalent (the sequencer blocks on each in turn).
    ctr = 0
    for fn in nc.m.functions:
        for bb in fn.blocks:
            insts = bb.instructions
            if not any(
                i.sync_info is not None and len(i.sync_info.on_wait) > max_waits
                for i in insts
            ):
                continue
            new_list = []
            for ins in insts:
                si = ins.sync_info
                if si is not None and len(si.on_wait) > max_waits:
                    waits = list(si.on_wait)
                    head, tail = waits[:-max_waits], waits[-max_waits:]
                    for k in range(0, len(head), max_waits):
                        nop = mybir.InstNoOp(
                            name=f"waitsplit-{ctr}",
                            engine=ins.engine,
                            ins=[],
                            outs=[],
                            sync_info=mybir.SyncInfo(
                                on_wait=head[k : k + max_waits], on_update=[]
                            ),
                        )
                        ctr += 1
                        new_list.append(nop)
                    ins.sync_info = mybir.SyncInfo(
                        on_wait=tail, on_update=list(si.on_update)
                    )
                new_list.append(ins)
            bb.instructions = new_list


def _build_nc():
    nc = bass.Bass("TRN2", target_bir_lowering=False, debug=False)
    h = nc.dram_tensor("h", [B_SH, S, D], dt.float32, kind="ExternalInput").ap()
    W = nc.dram_tensor("W", [O_SH, D], dt.float32, kind="ExternalInput").ap()
    bias = nc.dram_tensor("bias", [O_SH], dt.float32, kind="ExternalInput").ap()
    coeff = nc.dram_tensor("coeff", [B_SH], dt.float32, kind="ExternalInput").ap()
    mask = nc.dram_tensor("mask", [B_SH, D, O_SH], dt.int32, kind="ExternalInput").ap()
    out = nc.dram_tensor("out", [B_SH, S, O_SH], dt.float32, kind="ExternalOutput").ap()

    with tile.TileContext(nc) as tc:
        with (
            tc.tile_pool(name="const", bufs=1) as const_pool,
            tc.tile_pool(name="wstage", bufs=5) as wstage_pool,  # W f32 staging
            tc.tile_pool(name="hstage", bufs=6) as hstage_pool,  # h bf16 staging
            tc.tile_pool(name="wt", bufs=KC + 4) as wt_pool,
            tc.tile_pool(name="ht", bufs=B_SH * KC) as ht_pool,
            tc.tile_pool(name="mk", bufs=12) as mk_pool,
            tc.tile_pool(name="tt", bufs=4) as t_pool,
            tc.tile_pool(name="m", bufs=KC + 4) as m_pool,
            tc.tile_pool(name="ost", bufs=3) as out_pool,
            tc.tile_pool(name="tp", bufs=3, space="PSUM") as tp_pool,
            tc.tile_pool(name="acc", bufs=4, space="PSUM") as acc_pool,
        ):
            ident = const_pool.tile([128, 128], dt.float32)
            masks.make_identity(nc, ident[:])
            ident_bf = const_pool.tile([128, 128], dt.bfloat16)
            masks.make_identity(nc, ident_bf[:])

            bias_bc = const_pool.tile([128, O_SH], dt.float32)
            nc.sync.dma_start(
                bias_bc[:], bass.AP(bias.tensor, 0, [[0, 128], [1, O_SH]])
            )
            coeff_bc = const_pool.tile([128, B_SH], dt.float32)
            nc.sync.dma_start(
                coeff_bc[:], bass.AP(coeff.tensor, 0, [[0, 128], [1, B_SH]])
            )
            c2 = const_pool.tile([128, B_SH], dt.float32)
            cneg = const_pool.tile([128, B_SH], dt.float32)
            nc.vector.tensor_scalar_mul(c2[:], coeff_bc[:], 2.0)
            nc.vector.tensor_scalar_mul(cneg[:], coeff_bc[:], -1.0)

            ht = {}

            def build_ht_kg(b, kg):
                # h [b, s, i] -> (b, kc) tiles [128 i, 512 s] bf16, kc in kg*8..
                # f32 -> bf16 cast happens in the (SWDGE) DMA itself
                stg = [
                    hstage_pool.tile([128, 1024], dt.bfloat16, tag="hstage", name="hstg")
                    for _ in range(SC)
                ]
                for sc in range(SC):
                    nc.gpsimd.dma_start(
                        stg[sc][:],
                        h[b, sc * 128 : (sc + 1) * 128, kg * 1024 : (kg + 1) * 1024],
                    )
                for k8 in range(8):
                    kc = kg * 8 + k8
                    tp = tp_pool.tile([128, 512], dt.bfloat16, name="tp", tag="tp")
                    for sc in range(SC):
                        nc.tensor.transpose(
                            tp[:, sc * 128 : (sc + 1) * 128],
                            stg[sc][:, k8 * 128 : (k8 + 1) * 128],
                            ident_bf[:],
                        )
                    htt = ht_pool.tile([128, S], dt.bfloat16, name="htt")
                    nc.vector.tensor_copy(htt[:], tp[:])
                    ht[(b, kc)] = htt

            def build_wt_kg(half, kg, wt):
                # W.T chunks [128 i, 512 o] bf16 for kc in kg*8..
                o0 = half * HALF
                stg = [
                    wstage_pool.tile([128, 1024], dt.float32, tag="wstage", name="wstg")
                    for _ in range(4)
                ]
                for oc in range(4):
                    nc.sync.dma_start(
                        stg[oc][:],
                        W[
                            o0 + oc * 128 : o0 + (oc + 1) * 128,
                            kg * 1024 : (kg + 1) * 1024,
                        ],
                    )
                for k8 in range(8):
                    kc = kg * 8 + k8
                    tp = tp_pool.tile([128, 512], dt.float32, name="tp", tag="tp")
                    for oc in range(4):
                        nc.tensor.transpose(
                            tp[:, oc * 128 : (oc + 1) * 128],
                            stg[oc][:, k8 * 128 : (k8 + 1) * 128],
                            ident[:],
                        )
                    wtt = wt_pool.tile([128, HALF], dt.bfloat16, name="wtt")
                    nc.vector.tensor_copy(wtt[:], tp[:])  # DVE: PSUM f32 -> SBUF bf16
                    wt.append(wtt)

            def round_kg(half, b, kg, wt, accs):
                o0 = half * HALF
                for k8 in range(8):
                    kc = kg * 8 + k8
                    mk = mk_pool.tile([128, HALF], dt.int32, name="mk")
                    nc.sync.dma_start(
                        mk[:], mask[b, kc * 128 : (kc + 1) * 128, o0 : o0 + HALF]
                    )
                    t_sb = t_pool.tile([128, HALF], dt.bfloat16, name="tsb")
                    nc.scalar.activation(
                        t_sb[:],
                        mk[:],
                        mybir.ActivationFunctionType.Identity,
                        bias=cneg[:, b : b + 1],
                        scale=c2[:, b : b + 1],
                    )
                    m = m_pool.tile([128, HALF], dt.bfloat16, name="m")
                    nc.vector.tensor_tensor(
                        m[:], t_sb[:], wt[kc][:], mybir.AluOpType.add
                    )
                    for sc in range(SC):
                        nc.tensor.matmul(
                            accs[sc][:],
                            ht[(b, kc)][:, sc * 128 : (sc + 1) * 128],
                            m[:],
                            start=(kc == 0),
                            stop=(kc == KC - 1),
                        )

            def epilogue(half, b, accs):
                o0 = half * HALF
                for sc in range(SC):
                    o_sb = out_pool.tile([128, HALF], dt.float32, name="osb")
                    nc.vector.tensor_tensor(
                        o_sb[:],
                        accs[sc][:],
                        bias_bc[:, o0 : o0 + HALF],
                        mybir.AluOpType.add,
                    )
                    nc.sync.dma_start(
                        out[b, sc * 128 : (sc + 1) * 128, o0 : o0 + HALF], o_sb[:]
                    )

            def new_accs():
                return [
                    acc_pool.tile([128, HALF], dt.float32, tag="acc", name="acc")
                    for _ in range(SC)
                ]

            wt0, wt1 = [], []
            # (half 0, b 0): build hT(b0)/WT(h0)/hT(b1) interleaved
            accs = new_accs()
            for kg in range(4):
                build_ht_kg(0, kg)
                build_wt_kg(0, kg, wt0)
                round_kg(0, 0, kg, wt0, accs)
                build_ht_kg(1, kg)
            epilogue(0, 0, accs)
            # (half 0, b 1): build WT(h1) interleaved (wt1[kc] slots reuse
            # wt0[kc] right after its last read in this round)
            accs = new_accs()
            for kg in range(4):
                round_kg(0, 1, kg, wt0, accs)
                build_wt_kg(1, kg, wt1)
            epilogue(0, 1, accs)
            accs = new_accs()
            for kg in range(4):
                round_kg(1, 0, kg, wt1, accs)
            epilogue(1, 0, accs)
            accs = new_accs()
            for kg in range(4):
                round_kg(1, 1, kg, wt1, accs)
            epilogue(1, 1, accs)

    _split_sync_waits(nc)
    return nc


def _get_nc():
    if "nc" not in _CACHE:
        _CACHE["nc"] = _build_nc()
    return _CACHE["nc"]


def kernel(hidden_states, W, b, coeff, mask, _trace=False, _trace_kwargs=None):
    nc = _get_nc()
    in_maps = []
    for core in range(N_CORES):
        g, bj = core // BG, core % BG
        in_maps.append(
            {
                "h": np.ascontiguousarray(
                    hidden_states[bj * B_SH : (bj + 1) * B_SH], dtype=np.float32
                ),
                "W": np.ascontiguousarray(
                    W[g * O_SH : (g + 1) * O_SH], dtype=np.float32
                ),
                "bias": np.ascontiguousarray(
                    b[g * O_SH : (g + 1) * O_SH], dtype=np.float32
                ),
                "coeff": np.ascontiguousarray(
                    coeff[bj * B_SH : (bj + 1) * B_SH], dtype=np.float32
                ),
                "mask": np.ascontiguousarray(
                    mask[bj * B_SH : (bj + 1) * B_SH, :, g * O_SH : (g + 1) * O_SH],
                    dtype=np.int32,
                ),
            }
        )
    kwargs = {}
    if _trace:
        kwargs = {"trace": True, "trace_kwargs": _trace_kwargs or {}}
    res = run_bass_kernel_spmd(nc, in_maps, core_ids=list(range(N_CORES)), **kwargs)
    _CACHE["last_results"] = res

    out = np.empty((B, S, O_FULL), dtype=np.float32)
    for core in range(N_CORES):
        g, bj = core // BG, core % BG
        out[bj * B_SH : (bj + 1) * B_SH, :, g * O_SH : (g + 1) * O_SH] = res.results[
            core
        ]["out"]
    return out
